# revision 13
# baseline (speedup 1.0000x reference)
"""DiagonalLSTM Bass/Tile kernel for TRN2 (per-core shard: B=4 images).

DESIGN-D ("b-inner"): state cols are 4*kap + b (batch innermost), row
parity u = p % 2 selects 128-col blocks. This makes the in-band kap
ranges contiguous and, critically, lets the four per-step sigmoids fuse
into two (one per parity u, spanning both channel-pair tiles) with
3-free-dim APs.

  - A2  [128,256] bf16: parts 0:64 h[k, col], parts 64:128 x[c, col];
    col = 128u + 4*kap + b, state row p = 2*kap + u.
  - P   [128,512] PSUM (one bank): P[64a+k, 256w + 128u + 4kap + b] =
    gate preactivation channel o = 64*(2w+a)+k for row p = 2kap+u.
    Bias (b_i2s+b_s2s) enters via two rank-1 matmuls that START the
    accumulation group, so the sigmoids need no bias operand.
  - G   [128,512] f32: G[64u'+k', 128q + 4kap' + b] = sigmoid(P) in the
    model's reinterpreted coords (quarter q, new row p' = 2kap'+u').
    sigma-u: in  v(P, 128u, [[256,2],[4,32],[1,4]])
             out v(G,   8u, [[4,2],[16,32],[1,4]])
  - The post-sigmoid tail splits by u (old-row parity = kap'%4 block):
    col pattern [[16,8],[1,8]] offset 128q + 8u. The u=0 half pipelines
    against sigma-u1.
  - c-state C2BF [128,128] bf16 (parts 64u_new+k, cols 4kap_new+b) is
    written directly by the ADDs; the c2c matmuls read its partition
    halves via duplicated lhsT copies (LCQ*) so no separate casts.
  - Upsample U + OUT bias-add for step t-1 run during step t (off the
    critical PE path). Output rows DMA to DRAM as soon as final
    (row p after step p+63), so there is no store tail.
"""
from contextlib import ExitStack

import numpy as np

import concourse.bass as bass
import concourse.tile as tile
from concourse import bacc, mybir

F32 = mybir.dt.float32
BF = mybir.dt.bfloat16
AF = mybir.ActivationFunctionType
ALU = mybir.AluOpType

B = 4          # images per core
H = 64         # rows
W = 64         # cols
C = 64         # input channels
HID = 64       # hidden
NW = H + W - 1 # 127 diagonal steps

DU = ((16, 8), (1, 8))  # u-subset col pattern (within one 128-col block)


def v(ap, off, dims):
    """Custom view: keep ap's partition dim, replace free dims, add offset
    (in elements)."""
    return bass.AP(ap.tensor, ap.offset + off, [list(ap.ap[0])] + [list(d) for d in dims])


def dv(ap, off, dims):
    """Fully-custom view (DRAM side of DMAs): absolute offset, all dims."""
    return bass.AP(ap.tensor, off, [list(d) for d in dims])


def band(t):
    return max(0, t - (W - 1)), min(H - 1, t)


def parity_band(t, u):
    """(kap0, n) for rows p in band(t) with p % 2 == u; n may be 0."""
    lo, hi = band(t)
    p0 = lo + ((u - lo) % 2)
    if p0 > hi:
        return 0, 0
    return (p0 - u) // 2, (hi - p0) // 2 + 1


def build_kernel(ctx, tc, outs, ins):
    nc = tc.nc
    x_d = ins["inputs"]
    out_d = outs["out"]

    const = ctx.enter_context(tc.tile_pool(name="const", bufs=1))
    big = ctx.enter_context(tc.tile_pool(name="big", bufs=1))
    st = ctx.enter_context(tc.tile_pool(name="st", bufs=2))
    tmp = ctx.enter_context(tc.tile_pool(name="tmp", bufs=2))
    ps = ctx.enter_context(tc.tile_pool(name="ps", bufs=2, space="PSUM"))

    # ---------------- weights / biases (one-time prep) ----------------
    LA0 = const.tile([128, 128], BF, tag="LA0")   # [[Ws1 o=0:128].T ; [Wi2s].T]
    LA1 = const.tile([128, 128], BF, tag="LA1")
    LB0 = const.tile([64, 128], BF, tag="LB0")    # Ws0.T per channel-pair tile
    LB1 = const.tile([64, 128], BF, tag="LB1")
    LC1 = const.tile([64, 64], BF, tag="LC1")     # Wc1.T
    LC0 = const.tile([64, 64], BF, tag="LC0")
    LU = const.tile([64, 128], BF, tag="LU")      # w_up.T
    LA0f = const.tile([128, 128], F32, tag="LA0f")
    LA1f = const.tile([128, 128], F32, tag="LA1f")
    LB0f = const.tile([64, 128], F32, tag="LB0f")
    LB1f = const.tile([64, 128], F32, tag="LB1f")
    LC1f = const.tile([64, 64], F32, tag="LC1f")
    LC0f = const.tile([64, 64], F32, tag="LC0f")
    LUf = const.tile([64, 128], F32, tag="LUf")
    LBI = const.tile([1, 256], F32, tag="LBI")
    LBS = const.tile([1, 256], F32, tag="LBS")
    LBSUM = const.tile([1, 256], F32, tag="LBSUM")
    LBIAS = const.tile([1, 256], BF, tag="LBIAS")
    ONES = const.tile([1, 256], BF, tag="ONES")
    bc2c2 = const.tile([128, 1], F32, tag="bc2c2")
    bup = const.tile([128, 1], F32, tag="bup")

    w_s2s = ins["w_s2s"]   # [256, 64, 2] dram
    w_i2s = ins["w_i2s"]   # [256, 64]
    w_c2c = ins["w_c2c"]   # [64, 64, 2]
    w_up = ins["w_up"]     # [128, 64]

    for blk, LA, LB in ((0, LA0f, LB0f), (1, LA1f, LB1f)):
        nc.sync.dma_start(
            out=LA[0:64, :],
            in_=dv(w_s2s, 128 * blk * 128 + 1, [[2, 64], [128, 128]]),
        )
        nc.sync.dma_start(
            out=LA[64:128, :],
            in_=dv(w_i2s, 128 * blk * 64, [[1, 64], [64, 128]]),
        )
        nc.sync.dma_start(
            out=LB[:, :],
            in_=dv(w_s2s, 128 * blk * 128 + 0, [[2, 64], [128, 128]]),
        )
    nc.sync.dma_start(out=LC1f[:, :], in_=dv(w_c2c, 1, [[2, 64], [128, 64]]))
    nc.sync.dma_start(out=LC0f[:, :], in_=dv(w_c2c, 0, [[2, 64], [128, 64]]))
    nc.sync.dma_start(out=LUf[:, :], in_=dv(w_up, 0, [[1, 64], [64, 128]]))
    for bf_t, f_t in ((LA0, LA0f), (LA1, LA1f), (LB0, LB0f), (LB1, LB1f),
                      (LC1, LC1f), (LC0, LC0f), (LU, LUf)):
        nc.vector.tensor_copy(bf_t[:, :], f_t[:, :])

    b_i2s, b_s2s, b_c2c, b_up = ins["b_i2s"], ins["b_s2s"], ins["b_c2c"], ins["b_up"]
    nc.sync.dma_start(out=LBI[:, :], in_=dv(b_i2s, 0, [[1, 1], [1, 256]]))
    nc.sync.dma_start(out=LBS[:, :], in_=dv(b_s2s, 0, [[1, 1], [1, 256]]))
    nc.vector.tensor_add(LBSUM[:, :], LBI[:, :], LBS[:, :])
    nc.vector.tensor_copy(LBIAS[:, :], LBSUM[:, :])
    nc.vector.memset(ONES[:, :], 1.0)
    nc.sync.dma_start(out=bc2c2[0:64, :], in_=dv(b_c2c, 0, [[1, 64], [1, 1]]))
    nc.sync.dma_start(out=bc2c2[64:128, :], in_=dv(b_c2c, 0, [[1, 64], [1, 1]]))
    nc.sync.dma_start(out=bup[:, :], in_=dv(b_up, 0, [[1, 128], [1, 1]]))

    # ---------------- input load ----------------
    # IN[c, b*4096 + p*64 + w] = inputs[b, c, p, w]
    IN = big.tile([64, B * H * W], BF, tag="IN")
    for b in range(B):
        nc.sync.dma_start(
            out=IN[:, b * H * W:(b + 1) * H * W],
            in_=dv(x_d, b * C * H * W, [[4096, 64], [1, 4096]]),
        )

    OUT = big.tile([128, B * H * W], F32, tag="OUT")
    IN_ap = IN[:, :]
    OUT_ap = OUT[:, :]

    def xprep(A2b, t):
        """Fill the x half (parts 64:128) of A2b for step t."""
        xa = A2b[64:128, :]
        nc.gpsimd.memset(xa, 0.0)
        for u in (0, 1):
            k0, n = parity_band(t, u)
            if n:
                nc.gpsimd.tensor_copy(
                    out=v(xa, u * 128 + 4 * k0, [[4, n], [1, 4]]),
                    in_=v(IN_ap, 63 * (2 * k0 + u) + t, [[126, n], [4096, 4]]),
                )



    def emit_cp(Cp, ce_t, co_t):
        """c2c matmuls: Cp[64un+k, 4kap+b] from rebased c halves (bf16,
        partition base 0 -- a base-64 lhsT in an accumulation group kills
        the hardware)."""
        ce, co = ce_t[:, :], co_t[:, :]
        # even rows p=2kap: Wc1*c[p] + Wc0*c[p-1] (odd, kap-1)
        nc.tensor.matmul(Cp[0:64, :], LC1[:, :], ce,
                         start=True, stop=False, skip_group_check=True)
        nc.tensor.matmul(v(Cp[0:64, :], 4, [[1, 124]]),
                         LC0[:, :], v(co, 0, [[1, 124]]),
                         start=False, stop=True, skip_group_check=True)
        # odd rows p=2kap+1: Wc1*c[p] + Wc0*c[p-1] (even, same kap)
        nc.tensor.matmul(Cp[64:128, :], LC1[:, :], co,
                         start=True, stop=False, skip_group_check=True)
        nc.tensor.matmul(Cp[64:128, :], LC0[:, :], ce,
                         start=False, stop=True, skip_group_check=True)

    # ---------------- initial state ----------------
    A2 = st.tile([128, 256], BF, tag="A2", name="A2", bufs=3)
    nc.gpsimd.memset(A2[0:64, :], 0.0)
    xprep(A2, 0)
    C2e = st.tile([64, 128], BF, tag="C2e", bufs=3)
    nc.gpsimd.memset(C2e[:, :], 0.0)
    C2o = st.tile([64, 128], BF, tag="C2o", bufs=3)
    nc.gpsimd.memset(C2o[:, :], 0.0)

    Cp = ps.tile([128, 128], F32, tag="Cp", name="Cp")
    emit_cp(Cp, C2e, C2o)

    # ---------------- the recurrence ----------------
    for t in range(NW):
        P = ps.tile([128, 512], F32, tag="P", name="P")
        Pap = P[:, :]
        A2h = A2[0:64, :]

        # gate matmuls, u=0 first so sigma-u0 can start early. Each region's
        # rank-1 bias MM sits directly before its A-MM: an intervening MM to
        # another region closes the accumulation group and drops the bias.
        for u in (0, 1):
            for w, LA, LB in ((0, LA0, LB0), (1, LA1, LB1)):
                base = 256 * u + 128 * w
                nc.tensor.matmul(
                    Pap[:, base:base + 128],
                    LBIAS[:, 128 * w:128 * w + 128], ONES[:, 0:128],
                    start=True, stop=False, skip_group_check=True,
                )
                nc.tensor.matmul(
                    Pap[:, base:base + 128], LA[:, :], A2[:, 128 * u:128 * u + 128],
                    start=False, stop=False, skip_group_check=True,
                )
                if u == 1:
                    # out p odd <- h[p-1] (even, same kap)
                    nc.tensor.matmul(
                        Pap[:, base:base + 128], LB[:, :], A2[0:64, 0:128],
                        start=False, stop=True, skip_group_check=True,
                    )
                else:
                    # out p even, kap >= 1 <- h[p-1] (odd, kap-1)
                    nc.tensor.matmul(
                        v(Pap, base + 4, [[1, 124]]),
                        LB[:, :], v(A2h, 128, [[1, 124]]),
                        start=False, stop=True, skip_group_check=True,
                    )

        # upsample + OUT for step t-1 (reads A2 h half = h_{t-1})
        U = None
        if t > 0:
            U = ps.tile([128, 256], F32, tag="U", name="U")
            for u in (0, 1):
                k0, n = parity_band(t - 1, u)
                if n:
                    nc.tensor.matmul(
                        v(U[:, :], 128 * u + 4 * k0, [[1, 4 * n]]),
                        LU[:, :], v(A2h, 128 * u + 4 * k0, [[1, 4 * n]]),
                        start=True, stop=True, skip_group_check=True,
                    )

        # x for step t+1
        A2n = st.tile([128, 256], BF, tag="A2", name="A2n", bufs=3)
        if t + 1 < NW:
            xprep(A2n, t + 1)

        # fused sigmoids: one per parity u, spanning both channel tiles
        G = tmp.tile([128, 512], F32, tag="G")
        Gap = G[:, :]
        for u in (0, 1):
            nc.scalar.activation(
                v(Gap, 8 * u, [[4, 2], [16, 32], [1, 4]]),
                v(Pap, 256 * u, [[128, 2], [4, 32], [1, 4]]),
                AF.Sigmoid,
            )

        # gate math, pipelined by parity u across vector/gpsimd/scalar
        T1 = tmp.tile([128, 128], F32, tag="T1")
        T2 = tmp.tile([128, 128], F32, tag="T2")
        C2n = st.tile([128, 128], BF, tag="C2BF", bufs=3)
        TH = tmp.tile([128, 128], F32, tag="TH")
        Cpap = Cp[:, :]
        T1ap, T2ap, C2nap, THap = T1[:, :], T2[:, :], C2n[:, :], TH[:, :]

        # queue-order matters: vector [T2a,ADDa,T2b,ADDb,MUL00,MUL01],
        # gpsimd [T1a,T1b,MUL10,MUL11], scalar [sig0,sig1,TANHa,TANHb] --
        # the u=1 chain pipelines behind u=0 without queue blocking.
        # (gpsimd cannot access PSUM, so T2/OUT stay on vector.)
        for u in (0, 1):
            o = 8 * u
            nc.vector.tensor_mul(
                v(T1ap, o, DU), v(Gap, 0 + o, DU), v(Gap, 128 + o, DU))
            nc.vector.scalar_tensor_tensor(
                out=v(T2ap, o, DU), in0=v(Cpap, o, DU), scalar=bc2c2[:, 0:1],
                in1=v(Gap, 256 + o, DU), op0=ALU.add, op1=ALU.mult)
            nc.vector.tensor_add(v(C2nap, o, DU), v(T1ap, o, DU), v(T2ap, o, DU))
        for u in (0, 1):
            o = 8 * u
            nc.scalar.activation(v(THap, o, DU), v(C2nap, o, DU), AF.Tanh)
        # h = og * tanh(c) into A2n h-half quadrants
        for un in (0, 1):
            for u in (0, 1):
                o = 8 * u
                eng = nc.vector if un == 0 else nc.gpsimd
                eng.tensor_mul(
                    v(A2n[0:64, :], 128 * un + o, DU),
                    v(G[64 * un:64 * un + 64, :], 384 + o, DU),
                    v(TH[64 * un:64 * un + 64, :], o, DU),
                )

        # rebase c halves to partition 0 (vector), then c2c matmuls for t+1
        C2en = st.tile([64, 128], BF, tag="C2e", bufs=3)
        nc.vector.tensor_copy(C2en[:, :], C2n[0:64, :])
        C2on = st.tile([64, 128], BF, tag="C2o", bufs=3)
        nc.vector.tensor_copy(C2on[:, :], C2n[64:128, :])
        Cpn = ps.tile([128, 128], F32, tag="Cp", name="Cpn")
        emit_cp(Cpn, C2en, C2on)

        # OUT bias-add for step t-1 (vector, end of queue) + early row DMA
        if t > 0:
            for u in (0, 1):
                k0, n = parity_band(t - 1, u)
                if n:
                    nc.vector.tensor_scalar_add(
                        v(OUT_ap, 63 * (2 * k0 + u) + (t - 1), [[4096, 4], [126, n]]),
                        v(U[:, :], 128 * u + 4 * k0, [[1, 4], [4, n]]),
                        bup[:, 0:1],
                    )
        p_done = t - 64
        if p_done >= 0:
            nc.sync.dma_start(
                out=dv(out_d, p_done * 64, [[4096, 128], [524288, 4], [1, 64]]),
                in_=v(OUT_ap, p_done * 64, [[4096, 4], [1, 64]]),
            )

        A2 = A2n
        Cp = Cpn

    # ---------------- epilogue: upsample/store for t = NW-1 ----------------
    t1 = NW - 1
    A2h = A2[0:64, :]
    U = ps.tile([128, 256], F32, tag="U", name="Ulast")
    for u in (0, 1):
        k0, n = parity_band(t1, u)
        if n:
            nc.tensor.matmul(
                v(U[:, :], 128 * u + 4 * k0, [[1, 4 * n]]),
                LU[:, :], v(A2h, 128 * u + 4 * k0, [[1, 4 * n]]),
                start=True, stop=True, skip_group_check=True,
            )
    for u in (0, 1):
        k0, n = parity_band(t1, u)
        if n:
            nc.vector.tensor_scalar_add(
                v(OUT_ap, 63 * (2 * k0 + u) + t1, [[4096, 4], [126, n]]),
                v(U[:, :], 128 * u + 4 * k0, [[1, 4], [4, n]]),
                bup[:, 0:1],
            )
    nc.sync.dma_start(
        out=dv(out_d, 63 * 64, [[4096, 128], [524288, 4], [1, 64]]),
        in_=v(OUT_ap, 63 * 64, [[4096, 4], [1, 64]]),
    )


def build_nc():
    nc = bacc.Bacc("TRN2", target_bir_lowering=False, debug=False)
    ins = {
        "inputs": nc.dram_tensor("inputs", [B, C, H, W], BF, kind="ExternalInput").ap(),
        "w_i2s": nc.dram_tensor("w_i2s", [4 * HID, C], F32, kind="ExternalInput").ap(),
        "b_i2s": nc.dram_tensor("b_i2s", [4 * HID], F32, kind="ExternalInput").ap(),
        "w_s2s": nc.dram_tensor("w_s2s", [4 * HID, HID, 2], F32, kind="ExternalInput").ap(),
        "b_s2s": nc.dram_tensor("b_s2s", [4 * HID], F32, kind="ExternalInput").ap(),
        "w_c2c": nc.dram_tensor("w_c2c", [HID, HID, 2], F32, kind="ExternalInput").ap(),
        "b_c2c": nc.dram_tensor("b_c2c", [HID], F32, kind="ExternalInput").ap(),
        "w_up": nc.dram_tensor("w_up", [2 * HID, HID], F32, kind="ExternalInput").ap(),
        "b_up": nc.dram_tensor("b_up", [2 * HID], F32, kind="ExternalInput").ap(),
    }
    outs = {"out": nc.dram_tensor("out", [B, 2 * HID, H, W], F32, kind="ExternalOutput").ap()}
    with tile.TileContext(nc) as tc:
        with ExitStack() as ctx:
            build_kernel(ctx, tc, outs, ins)
    nc.compile()
    return nc


# ---------------------------------------------------------------------------
# Harness entry point: full inputs -> shard over 8 cores -> full output.
# ---------------------------------------------------------------------------
import ml_dtypes
from concourse.bass_utils import run_bass_kernel_spmd

N_CORES = 8
TRACE = False
LAST_EXEC_NS = None
_NC = None


def _get_nc():
    global _NC
    if _NC is None:
        _NC = build_nc()
    return _NC


def kernel(**inputs):
    global LAST_EXEC_NS
    nc = _get_nc()
    full = {k: np.ascontiguousarray(np.asarray(val, np.float32))
            for k, val in inputs.items()}
    xs = full["inputs"].astype(ml_dtypes.bfloat16)
    in_maps = []
    for i in range(N_CORES):
        m = dict(full)
        m["inputs"] = np.ascontiguousarray(xs[B * i:B * (i + 1)])
        in_maps.append(m)
    res = run_bass_kernel_spmd(nc, in_maps, list(range(N_CORES)), trace=TRACE)
    LAST_EXEC_NS = res.exec_time_ns
    return np.concatenate([res.results[i]["out"] for i in range(N_CORES)], axis=0)


# revision 15
# speedup vs baseline: 1.1518x; 1.1518x over previous
"""DiagonalLSTM Bass/Tile kernel for TRN2 (per-core shard: B=4 images).

DESIGN-D ("b-inner"): state cols are 4*kap + b (batch innermost), row
parity u = p % 2 selects 128-col blocks. This makes the in-band kap
ranges contiguous and, critically, lets the four per-step sigmoids fuse
into two (one per parity u, spanning both channel-pair tiles) with
3-free-dim APs.

  - A2  [128,256] bf16: parts 0:64 h[k, col], parts 64:128 x[c, col];
    col = 128u + 4*kap + b, state row p = 2*kap + u.
  - P   [128,512] PSUM (one bank): P[64a+k, 256w + 128u + 4kap + b] =
    gate preactivation channel o = 64*(2w+a)+k for row p = 2kap+u.
    Bias (b_i2s+b_s2s) enters via two rank-1 matmuls that START the
    accumulation group, so the sigmoids need no bias operand.
  - G   [128,512] f32: G[64u'+k', 128q + 4kap' + b] = sigmoid(P) in the
    model's reinterpreted coords (quarter q, new row p' = 2kap'+u').
    sigma-u: in  v(P, 128u, [[256,2],[4,32],[1,4]])
             out v(G,   8u, [[4,2],[16,32],[1,4]])
  - The post-sigmoid tail splits by u (old-row parity = kap'%4 block):
    col pattern [[16,8],[1,8]] offset 128q + 8u. The u=0 half pipelines
    against sigma-u1.
  - c-state C2BF [128,128] bf16 (parts 64u_new+k, cols 4kap_new+b) is
    written directly by the ADDs; the c2c matmuls read its partition
    halves via duplicated lhsT copies (LCQ*) so no separate casts.
  - Upsample U + OUT bias-add for step t-1 run during step t (off the
    critical PE path). Output rows DMA to DRAM as soon as final
    (row p after step p+63), so there is no store tail.
"""
from contextlib import ExitStack

import numpy as np

import concourse.bass as bass
import concourse.tile as tile
from concourse import bacc, mybir

F32 = mybir.dt.float32
BF = mybir.dt.bfloat16
AF = mybir.ActivationFunctionType
ALU = mybir.AluOpType

B = 4          # images per core
H = 64         # rows
W = 64         # cols
C = 64         # input channels
HID = 64       # hidden
NW = H + W - 1 # 127 diagonal steps

DU = ((16, 8), (1, 8))  # u-subset col pattern (within one 128-col block)


def v(ap, off, dims):
    """Custom view: keep ap's partition dim, replace free dims, add offset
    (in elements)."""
    return bass.AP(ap.tensor, ap.offset + off, [list(ap.ap[0])] + [list(d) for d in dims])


def dv(ap, off, dims):
    """Fully-custom view (DRAM side of DMAs): absolute offset, all dims."""
    return bass.AP(ap.tensor, off, [list(d) for d in dims])


def band(t):
    return max(0, t - (W - 1)), min(H - 1, t)


def parity_band(t, u):
    """(kap0, n) for rows p in band(t) with p % 2 == u; n may be 0."""
    lo, hi = band(t)
    p0 = lo + ((u - lo) % 2)
    if p0 > hi:
        return 0, 0
    return (p0 - u) // 2, (hi - p0) // 2 + 1


def build_kernel(ctx, tc, outs, ins):
    nc = tc.nc
    x_d = ins["inputs"]
    out_d = outs["out"]

    const = ctx.enter_context(tc.tile_pool(name="const", bufs=1))
    big = ctx.enter_context(tc.tile_pool(name="big", bufs=1))
    st = ctx.enter_context(tc.tile_pool(name="st", bufs=2))
    tmp = ctx.enter_context(tc.tile_pool(name="tmp", bufs=2))
    ps = ctx.enter_context(tc.tile_pool(name="ps", bufs=2, space="PSUM"))

    # ---------------- weights / biases (one-time prep) ----------------
    LA0 = const.tile([128, 128], BF, tag="LA0")   # [[Ws1 o=0:128].T ; [Wi2s].T]
    LA1 = const.tile([128, 128], BF, tag="LA1")
    LB0 = const.tile([64, 128], BF, tag="LB0")    # Ws0.T per channel-pair tile
    LB1 = const.tile([64, 128], BF, tag="LB1")
    LC1 = const.tile([64, 64], BF, tag="LC1")     # Wc1.T
    LC0 = const.tile([64, 64], BF, tag="LC0")
    LU = const.tile([64, 128], BF, tag="LU")      # w_up.T
    LA0f = const.tile([128, 128], F32, tag="LA0f")
    LA1f = const.tile([128, 128], F32, tag="LA1f")
    LB0f = const.tile([64, 128], F32, tag="LB0f")
    LB1f = const.tile([64, 128], F32, tag="LB1f")
    LC1f = const.tile([64, 64], F32, tag="LC1f")
    LC0f = const.tile([64, 64], F32, tag="LC0f")
    LUf = const.tile([64, 128], F32, tag="LUf")
    LBI = const.tile([1, 256], F32, tag="LBI")
    LBS = const.tile([1, 256], F32, tag="LBS")
    LBSUM = const.tile([1, 256], F32, tag="LBSUM")
    LBIAS = const.tile([1, 256], BF, tag="LBIAS")
    ONES = const.tile([1, 256], BF, tag="ONES")
    bc2c2 = const.tile([128, 1], F32, tag="bc2c2")
    bup = const.tile([128, 1], F32, tag="bup")

    w_s2s = ins["w_s2s"]   # [256, 64, 2] dram
    w_i2s = ins["w_i2s"]   # [256, 64]
    w_c2c = ins["w_c2c"]   # [64, 64, 2]
    w_up = ins["w_up"]     # [128, 64]

    for blk, LA, LB in ((0, LA0f, LB0f), (1, LA1f, LB1f)):
        nc.sync.dma_start(
            out=LA[0:64, :],
            in_=dv(w_s2s, 128 * blk * 128 + 1, [[2, 64], [128, 128]]),
        )
        nc.sync.dma_start(
            out=LA[64:128, :],
            in_=dv(w_i2s, 128 * blk * 64, [[1, 64], [64, 128]]),
        )
        nc.sync.dma_start(
            out=LB[:, :],
            in_=dv(w_s2s, 128 * blk * 128 + 0, [[2, 64], [128, 128]]),
        )
    nc.sync.dma_start(out=LC1f[:, :], in_=dv(w_c2c, 1, [[2, 64], [128, 64]]))
    nc.sync.dma_start(out=LC0f[:, :], in_=dv(w_c2c, 0, [[2, 64], [128, 64]]))
    nc.sync.dma_start(out=LUf[:, :], in_=dv(w_up, 0, [[1, 64], [64, 128]]))
    for bf_t, f_t in ((LA0, LA0f), (LA1, LA1f), (LB0, LB0f), (LB1, LB1f),
                      (LC1, LC1f), (LC0, LC0f), (LU, LUf)):
        nc.vector.tensor_copy(bf_t[:, :], f_t[:, :])

    b_i2s, b_s2s, b_c2c, b_up = ins["b_i2s"], ins["b_s2s"], ins["b_c2c"], ins["b_up"]
    nc.sync.dma_start(out=LBI[:, :], in_=dv(b_i2s, 0, [[1, 1], [1, 256]]))
    nc.sync.dma_start(out=LBS[:, :], in_=dv(b_s2s, 0, [[1, 1], [1, 256]]))
    nc.vector.tensor_add(LBSUM[:, :], LBI[:, :], LBS[:, :])
    nc.vector.tensor_copy(LBIAS[:, :], LBSUM[:, :])
    nc.vector.memset(ONES[:, :], 1.0)
    nc.sync.dma_start(out=bc2c2[0:64, :], in_=dv(b_c2c, 0, [[1, 64], [1, 1]]))
    nc.sync.dma_start(out=bc2c2[64:128, :], in_=dv(b_c2c, 0, [[1, 64], [1, 1]]))
    nc.sync.dma_start(out=bup[:, :], in_=dv(b_up, 0, [[1, 128], [1, 1]]))

    # ---------------- input load ----------------
    # IN[c, b*4096 + p*64 + w] = inputs[b, c, p, w]
    IN = big.tile([64, B * H * W], BF, tag="IN")
    for b in range(B):
        nc.sync.dma_start(
            out=IN[:, b * H * W:(b + 1) * H * W],
            in_=dv(x_d, b * C * H * W, [[4096, 64], [1, 4096]]),
        )

    OUT = big.tile([128, B * H * W], F32, tag="OUT")
    IN_ap = IN[:, :]
    OUT_ap = OUT[:, :]

    def xprep(A2b, t):
        """Fill the x half (parts 64:128) of A2b for step t."""
        xa = A2b[64:128, :]
        nc.gpsimd.memset(xa, 0.0)
        for u in (0, 1):
            k0, n = parity_band(t, u)
            if n:
                nc.gpsimd.tensor_copy(
                    out=v(xa, u * 128 + 4 * k0, [[4, n], [1, 4]]),
                    in_=v(IN_ap, 63 * (2 * k0 + u) + t, [[126, n], [4096, 4]]),
                )



    def emit_cp(Cp, ce_t, co_t):
        """c2c matmuls: Cp[64un+k, 4kap+b] from rebased c halves (bf16,
        partition base 0 -- a base-64 lhsT in an accumulation group kills
        the hardware)."""
        ce, co = ce_t[:, :], co_t[:, :]
        # even rows p=2kap: Wc1*c[p] + Wc0*c[p-1] (odd, kap-1)
        nc.tensor.matmul(Cp[0:64, :], LC1[:, :], ce,
                         start=True, stop=False, skip_group_check=True)
        nc.tensor.matmul(v(Cp[0:64, :], 4, [[1, 124]]),
                         LC0[:, :], v(co, 0, [[1, 124]]),
                         start=False, stop=True, skip_group_check=True)
        # odd rows p=2kap+1: Wc1*c[p] + Wc0*c[p-1] (even, same kap)
        nc.tensor.matmul(Cp[64:128, :], LC1[:, :], co,
                         start=True, stop=False, skip_group_check=True)
        nc.tensor.matmul(Cp[64:128, :], LC0[:, :], ce,
                         start=False, stop=True, skip_group_check=True)

    # ---------------- initial state ----------------
    A2 = st.tile([128, 256], BF, tag="A2", name="A2", bufs=3)
    nc.gpsimd.memset(A2[0:64, :], 0.0)
    xprep(A2, 0)
    C2e = st.tile([64, 128], BF, tag="C2e", bufs=3)
    nc.gpsimd.memset(C2e[:, :], 0.0)
    C2o = st.tile([64, 128], BF, tag="C2o", bufs=3)
    nc.gpsimd.memset(C2o[:, :], 0.0)

    Cp = ps.tile([128, 128], F32, tag="Cp", name="Cp")
    emit_cp(Cp, C2e, C2o)

    # ---------------- the recurrence ----------------
    for t in range(NW):
        P0 = ps.tile([128, 256], F32, tag="P0", name="P0", padded_shape=[128, 512])
        P1 = ps.tile([128, 256], F32, tag="P1", name="P1", padded_shape=[128, 512])
        Pu = (P0[:, :], P1[:, :])
        A2h = A2[0:64, :]

        # gate matmuls, u=0 first so sigma-u0 can start early. Each region's
        # rank-1 bias MM sits directly before its A-MM: an intervening MM to
        # another region closes the accumulation group and drops the bias.
        for u in (0, 1):
            Pap = Pu[u]
            for w, LA, LB in ((0, LA0, LB0), (1, LA1, LB1)):
                base = 128 * w
                nc.tensor.matmul(
                    Pap[:, base:base + 128],
                    LBIAS[:, 128 * w:128 * w + 128], ONES[:, 0:128],
                    start=True, stop=False, skip_group_check=True,
                )
                nc.tensor.matmul(
                    Pap[:, base:base + 128], LA[:, :], A2[:, 128 * u:128 * u + 128],
                    start=False, stop=False, skip_group_check=True,
                )
                if u == 1:
                    # out p odd <- h[p-1] (even, same kap)
                    nc.tensor.matmul(
                        Pap[:, base:base + 128], LB[:, :], A2[0:64, 0:128],
                        start=False, stop=True, skip_group_check=True,
                    )
                else:
                    # out p even, kap >= 1 <- h[p-1] (odd, kap-1)
                    nc.tensor.matmul(
                        v(Pap, base + 4, [[1, 124]]),
                        LB[:, :], v(A2h, 128, [[1, 124]]),
                        start=False, stop=True, skip_group_check=True,
                    )

        # upsample + OUT for step t-1 (reads A2 h half = h_{t-1})
        U = None
        if t > 0:
            U = ps.tile([128, 256], F32, tag="U", name="U")
            for u in (0, 1):
                k0, n = parity_band(t - 1, u)
                if n:
                    nc.tensor.matmul(
                        v(U[:, :], 128 * u + 4 * k0, [[1, 4 * n]]),
                        LU[:, :], v(A2h, 128 * u + 4 * k0, [[1, 4 * n]]),
                        start=True, stop=True, skip_group_check=True,
                    )

        # x for step t+1
        A2n = st.tile([128, 256], BF, tag="A2", name="A2n", bufs=3)
        if t + 1 < NW:
            xprep(A2n, t + 1)

        # fused sigmoids: one per parity u, spanning both channel tiles
        G = tmp.tile([128, 512], F32, tag="G")
        Gap = G[:, :]
        for u in (0, 1):
            nc.scalar.activation(
                v(Gap, 8 * u, [[4, 2], [16, 32], [1, 4]]),
                v(Pu[u], 0, [[128, 2], [4, 32], [1, 4]]),
                AF.Sigmoid,
            )

        # gate math, pipelined by parity u across vector/gpsimd/scalar
        T1 = tmp.tile([128, 128], F32, tag="T1")
        T2 = tmp.tile([128, 128], F32, tag="T2")
        C2n = st.tile([128, 128], BF, tag="C2BF", bufs=3)
        TH = tmp.tile([128, 128], F32, tag="TH")
        Cpap = Cp[:, :]
        T1ap, T2ap, C2nap, THap = T1[:, :], T2[:, :], C2n[:, :], TH[:, :]

        # queue-order matters: vector [T2a,ADDa,T2b,ADDb,MUL00,MUL01],
        # gpsimd [T1a,T1b,MUL10,MUL11], scalar [sig0,sig1,TANHa,TANHb] --
        # the u=1 chain pipelines behind u=0 without queue blocking.
        # (gpsimd cannot access PSUM, so T2/OUT stay on vector.)
        for u in (0, 1):
            o = 8 * u
            nc.vector.tensor_mul(
                v(T1ap, o, DU), v(Gap, 0 + o, DU), v(Gap, 128 + o, DU))
            nc.vector.scalar_tensor_tensor(
                out=v(T2ap, o, DU), in0=v(Cpap, o, DU), scalar=bc2c2[:, 0:1],
                in1=v(Gap, 256 + o, DU), op0=ALU.add, op1=ALU.mult)
            nc.vector.tensor_add(v(C2nap, o, DU), v(T1ap, o, DU), v(T2ap, o, DU))
        for u in (0, 1):
            o = 8 * u
            nc.scalar.activation(v(THap, o, DU), v(C2nap, o, DU), AF.Tanh)
        # h = og * tanh(c) into A2n h-half quadrants
        for un in (0, 1):
            for u in (0, 1):
                o = 8 * u
                eng = nc.vector if un == 0 else nc.gpsimd
                eng.tensor_mul(
                    v(A2n[0:64, :], 128 * un + o, DU),
                    v(G[64 * un:64 * un + 64, :], 384 + o, DU),
                    v(TH[64 * un:64 * un + 64, :], o, DU),
                )

        # rebase c halves to partition 0 (vector), then c2c matmuls for t+1
        C2en = st.tile([64, 128], BF, tag="C2e", bufs=3)
        nc.vector.tensor_copy(C2en[:, :], C2n[0:64, :])
        C2on = st.tile([64, 128], BF, tag="C2o", bufs=3)
        nc.vector.tensor_copy(C2on[:, :], C2n[64:128, :])
        Cpn = ps.tile([128, 128], F32, tag="Cp", name="Cpn")
        emit_cp(Cpn, C2en, C2on)

        # OUT bias-add for step t-1 (vector, end of queue) + early row DMA
        if t > 0:
            for u in (0, 1):
                k0, n = parity_band(t - 1, u)
                if n:
                    nc.vector.tensor_scalar_add(
                        v(OUT_ap, 63 * (2 * k0 + u) + (t - 1), [[4096, 4], [126, n]]),
                        v(U[:, :], 128 * u + 4 * k0, [[1, 4], [4, n]]),
                        bup[:, 0:1],
                    )
        p_done = t - 64
        if p_done >= 0:
            nc.sync.dma_start(
                out=dv(out_d, p_done * 64, [[4096, 128], [524288, 4], [1, 64]]),
                in_=v(OUT_ap, p_done * 64, [[4096, 4], [1, 64]]),
            )

        A2 = A2n
        Cp = Cpn

    # ---------------- epilogue: upsample/store for t = NW-1 ----------------
    t1 = NW - 1
    A2h = A2[0:64, :]
    U = ps.tile([128, 256], F32, tag="U", name="Ulast")
    for u in (0, 1):
        k0, n = parity_band(t1, u)
        if n:
            nc.tensor.matmul(
                v(U[:, :], 128 * u + 4 * k0, [[1, 4 * n]]),
                LU[:, :], v(A2h, 128 * u + 4 * k0, [[1, 4 * n]]),
                start=True, stop=True, skip_group_check=True,
            )
    for u in (0, 1):
        k0, n = parity_band(t1, u)
        if n:
            nc.vector.tensor_scalar_add(
                v(OUT_ap, 63 * (2 * k0 + u) + t1, [[4096, 4], [126, n]]),
                v(U[:, :], 128 * u + 4 * k0, [[1, 4], [4, n]]),
                bup[:, 0:1],
            )
    nc.sync.dma_start(
        out=dv(out_d, 63 * 64, [[4096, 128], [524288, 4], [1, 64]]),
        in_=v(OUT_ap, 63 * 64, [[4096, 4], [1, 64]]),
    )


def build_nc():
    nc = bacc.Bacc("TRN2", target_bir_lowering=False, debug=False)
    ins = {
        "inputs": nc.dram_tensor("inputs", [B, C, H, W], BF, kind="ExternalInput").ap(),
        "w_i2s": nc.dram_tensor("w_i2s", [4 * HID, C], F32, kind="ExternalInput").ap(),
        "b_i2s": nc.dram_tensor("b_i2s", [4 * HID], F32, kind="ExternalInput").ap(),
        "w_s2s": nc.dram_tensor("w_s2s", [4 * HID, HID, 2], F32, kind="ExternalInput").ap(),
        "b_s2s": nc.dram_tensor("b_s2s", [4 * HID], F32, kind="ExternalInput").ap(),
        "w_c2c": nc.dram_tensor("w_c2c", [HID, HID, 2], F32, kind="ExternalInput").ap(),
        "b_c2c": nc.dram_tensor("b_c2c", [HID], F32, kind="ExternalInput").ap(),
        "w_up": nc.dram_tensor("w_up", [2 * HID, HID], F32, kind="ExternalInput").ap(),
        "b_up": nc.dram_tensor("b_up", [2 * HID], F32, kind="ExternalInput").ap(),
    }
    outs = {"out": nc.dram_tensor("out", [B, 2 * HID, H, W], F32, kind="ExternalOutput").ap()}
    with tile.TileContext(nc) as tc:
        with ExitStack() as ctx:
            build_kernel(ctx, tc, outs, ins)
    nc.compile()
    return nc


# ---------------------------------------------------------------------------
# Harness entry point: full inputs -> shard over 8 cores -> full output.
# ---------------------------------------------------------------------------
import ml_dtypes
from concourse.bass_utils import run_bass_kernel_spmd

N_CORES = 8
TRACE = False
LAST_EXEC_NS = None
_NC = None


def _get_nc():
    global _NC
    if _NC is None:
        _NC = build_nc()
    return _NC


def kernel(**inputs):
    global LAST_EXEC_NS
    nc = _get_nc()
    full = {k: np.ascontiguousarray(np.asarray(val, np.float32))
            for k, val in inputs.items()}
    xs = full["inputs"].astype(ml_dtypes.bfloat16)
    in_maps = []
    for i in range(N_CORES):
        m = dict(full)
        m["inputs"] = np.ascontiguousarray(xs[B * i:B * (i + 1)])
        in_maps.append(m)
    res = run_bass_kernel_spmd(nc, in_maps, list(range(N_CORES)), trace=TRACE)
    LAST_EXEC_NS = res.exec_time_ns
    return np.concatenate([res.results[i]["out"] for i in range(N_CORES)], axis=0)


# revision 17
# speedup vs baseline: 1.1773x; 1.0221x over previous
"""DiagonalLSTM Bass/Tile kernel for TRN2 (per-core shard: B=4 images).

DESIGN-D ("b-inner"): state cols are 4*kap + b (batch innermost), row
parity u = p % 2 selects 128-col blocks. This makes the in-band kap
ranges contiguous and, critically, lets the four per-step sigmoids fuse
into two (one per parity u, spanning both channel-pair tiles) with
3-free-dim APs.

  - A2  [128,256] bf16: parts 0:64 h[k, col], parts 64:128 x[c, col];
    col = 128u + 4*kap + b, state row p = 2*kap + u.
  - P   [128,512] PSUM (one bank): P[64a+k, 256w + 128u + 4kap + b] =
    gate preactivation channel o = 64*(2w+a)+k for row p = 2kap+u.
    Bias (b_i2s+b_s2s) enters via two rank-1 matmuls that START the
    accumulation group, so the sigmoids need no bias operand.
  - G   [128,512] f32: G[64u'+k', 128q + 4kap' + b] = sigmoid(P) in the
    model's reinterpreted coords (quarter q, new row p' = 2kap'+u').
    sigma-u: in  v(P, 128u, [[256,2],[4,32],[1,4]])
             out v(G,   8u, [[4,2],[16,32],[1,4]])
  - The post-sigmoid tail splits by u (old-row parity = kap'%4 block):
    col pattern [[16,8],[1,8]] offset 128q + 8u. The u=0 half pipelines
    against sigma-u1.
  - c-state C2BF [128,128] bf16 (parts 64u_new+k, cols 4kap_new+b) is
    written directly by the ADDs; the c2c matmuls read its partition
    halves via duplicated lhsT copies (LCQ*) so no separate casts.
  - Upsample U + OUT bias-add for step t-1 run during step t (off the
    critical PE path). Output rows DMA to DRAM as soon as final
    (row p after step p+63), so there is no store tail.
"""
from contextlib import ExitStack

import numpy as np

import concourse.bass as bass
import concourse.tile as tile
from concourse import bacc, mybir

F32 = mybir.dt.float32
BF = mybir.dt.bfloat16
AF = mybir.ActivationFunctionType
ALU = mybir.AluOpType

B = 4          # images per core
H = 64         # rows
W = 64         # cols
C = 64         # input channels
HID = 64       # hidden
NW = H + W - 1 # 127 diagonal steps

DU = ((16, 8), (1, 8))  # u-subset col pattern (within one 128-col block)


def v(ap, off, dims):
    """Custom view: keep ap's partition dim, replace free dims, add offset
    (in elements)."""
    return bass.AP(ap.tensor, ap.offset + off, [list(ap.ap[0])] + [list(d) for d in dims])


def dv(ap, off, dims):
    """Fully-custom view (DRAM side of DMAs): absolute offset, all dims."""
    return bass.AP(ap.tensor, off, [list(d) for d in dims])


def band(t):
    return max(0, t - (W - 1)), min(H - 1, t)


def parity_band(t, u):
    """(kap0, n) for rows p in band(t) with p % 2 == u; n may be 0."""
    lo, hi = band(t)
    p0 = lo + ((u - lo) % 2)
    if p0 > hi:
        return 0, 0
    return (p0 - u) // 2, (hi - p0) // 2 + 1


def build_kernel(ctx, tc, outs, ins):
    nc = tc.nc
    x_d = ins["inputs"]
    out_d = outs["out"]

    const = ctx.enter_context(tc.tile_pool(name="const", bufs=1))
    big = ctx.enter_context(tc.tile_pool(name="big", bufs=1))
    st = ctx.enter_context(tc.tile_pool(name="st", bufs=2))
    tmp = ctx.enter_context(tc.tile_pool(name="tmp", bufs=2))
    ps = ctx.enter_context(tc.tile_pool(name="ps", bufs=2, space="PSUM"))

    # ---------------- weights / biases (one-time prep) ----------------
    LA0 = const.tile([128, 128], BF, tag="LA0")   # [[Ws1 o=0:128].T ; [Wi2s].T]
    LA1 = const.tile([128, 128], BF, tag="LA1")
    LB0 = const.tile([64, 128], BF, tag="LB0")    # Ws0.T per channel-pair tile
    LB1 = const.tile([64, 128], BF, tag="LB1")
    LC1 = const.tile([64, 64], BF, tag="LC1")     # Wc1.T
    LC0 = const.tile([64, 64], BF, tag="LC0")
    LU = const.tile([64, 128], BF, tag="LU")      # w_up.T
    LA0f = const.tile([128, 128], F32, tag="LA0f")
    LA1f = const.tile([128, 128], F32, tag="LA1f")
    LB0f = const.tile([64, 128], F32, tag="LB0f")
    LB1f = const.tile([64, 128], F32, tag="LB1f")
    LC1f = const.tile([64, 64], F32, tag="LC1f")
    LC0f = const.tile([64, 64], F32, tag="LC0f")
    LUf = const.tile([64, 128], F32, tag="LUf")
    LBI = const.tile([1, 256], F32, tag="LBI")
    LBS = const.tile([1, 256], F32, tag="LBS")
    LBSUM = const.tile([1, 256], F32, tag="LBSUM")
    LBIAS = const.tile([1, 256], BF, tag="LBIAS")
    ONES = const.tile([1, 256], BF, tag="ONES")
    bc2c2 = const.tile([128, 1], F32, tag="bc2c2")
    bup = const.tile([128, 1], F32, tag="bup")

    w_s2s = ins["w_s2s"]   # [256, 64, 2] dram
    w_i2s = ins["w_i2s"]   # [256, 64]
    w_c2c = ins["w_c2c"]   # [64, 64, 2]
    w_up = ins["w_up"]     # [128, 64]

    for blk, LA, LB in ((0, LA0f, LB0f), (1, LA1f, LB1f)):
        nc.sync.dma_start(
            out=LA[0:64, :],
            in_=dv(w_s2s, 128 * blk * 128 + 1, [[2, 64], [128, 128]]),
        )
        nc.sync.dma_start(
            out=LA[64:128, :],
            in_=dv(w_i2s, 128 * blk * 64, [[1, 64], [64, 128]]),
        )
        nc.sync.dma_start(
            out=LB[:, :],
            in_=dv(w_s2s, 128 * blk * 128 + 0, [[2, 64], [128, 128]]),
        )
    nc.sync.dma_start(out=LC1f[:, :], in_=dv(w_c2c, 1, [[2, 64], [128, 64]]))
    nc.sync.dma_start(out=LC0f[:, :], in_=dv(w_c2c, 0, [[2, 64], [128, 64]]))
    nc.sync.dma_start(out=LUf[:, :], in_=dv(w_up, 0, [[1, 64], [64, 128]]))
    for bf_t, f_t in ((LA0, LA0f), (LA1, LA1f), (LB0, LB0f), (LB1, LB1f),
                      (LC1, LC1f), (LC0, LC0f), (LU, LUf)):
        nc.vector.tensor_copy(bf_t[:, :], f_t[:, :])

    b_i2s, b_s2s, b_c2c, b_up = ins["b_i2s"], ins["b_s2s"], ins["b_c2c"], ins["b_up"]
    nc.sync.dma_start(out=LBI[:, :], in_=dv(b_i2s, 0, [[1, 1], [1, 256]]))
    nc.sync.dma_start(out=LBS[:, :], in_=dv(b_s2s, 0, [[1, 1], [1, 256]]))
    nc.vector.tensor_add(LBSUM[:, :], LBI[:, :], LBS[:, :])
    nc.vector.tensor_copy(LBIAS[:, :], LBSUM[:, :])
    nc.vector.memset(ONES[:, :], 1.0)
    nc.sync.dma_start(out=bc2c2[0:64, :], in_=dv(b_c2c, 0, [[1, 64], [1, 1]]))
    nc.sync.dma_start(out=bc2c2[64:128, :], in_=dv(b_c2c, 0, [[1, 64], [1, 1]]))
    nc.sync.dma_start(out=bup[:, :], in_=dv(b_up, 0, [[1, 128], [1, 1]]))

    # ---------------- input load ----------------
    # IN[c, b*4096 + p*64 + w] = inputs[b, c, p, w]
    IN = big.tile([64, B * H * W], BF, tag="IN")
    for b in range(B):
        nc.sync.dma_start(
            out=IN[:, b * H * W:(b + 1) * H * W],
            in_=dv(x_d, b * C * H * W, [[4096, 64], [1, 4096]]),
        )

    OUT = big.tile([128, B * H * W], F32, tag="OUT")
    IN_ap = IN[:, :]
    OUT_ap = OUT[:, :]

    def xprep(A2b, t):
        """Fill the x half (parts 64:128) of A2b for step t."""
        xa = A2b[64:128, :]
        nc.gpsimd.memset(xa, 0.0)
        for u in (0, 1):
            k0, n = parity_band(t, u)
            if n:
                nc.gpsimd.tensor_copy(
                    out=v(xa, u * 128 + 4 * k0, [[4, n], [1, 4]]),
                    in_=v(IN_ap, 63 * (2 * k0 + u) + t, [[126, n], [4096, 4]]),
                )



    def emit_cp(Cp, ce_t, co_t):
        """c2c matmuls: Cp[64un+k, 4kap+b] from rebased c halves (bf16,
        partition base 0 -- a base-64 lhsT in an accumulation group kills
        the hardware)."""
        ce, co = ce_t[:, :], co_t[:, :]
        # even rows p=2kap: Wc1*c[p] + Wc0*c[p-1] (odd, kap-1)
        nc.tensor.matmul(Cp[0:64, :], LC1[:, :], ce,
                         start=True, stop=False, skip_group_check=True)
        nc.tensor.matmul(v(Cp[0:64, :], 4, [[1, 124]]),
                         LC0[:, :], v(co, 0, [[1, 124]]),
                         start=False, stop=True, skip_group_check=True)
        # odd rows p=2kap+1: Wc1*c[p] + Wc0*c[p-1] (even, same kap)
        nc.tensor.matmul(Cp[64:128, :], LC1[:, :], co,
                         start=True, stop=False, skip_group_check=True)
        nc.tensor.matmul(Cp[64:128, :], LC0[:, :], ce,
                         start=False, stop=True, skip_group_check=True)

    # ---------------- initial state ----------------
    A2 = st.tile([128, 256], BF, tag="A2", name="A2", bufs=3)
    nc.gpsimd.memset(A2[0:64, :], 0.0)
    xprep(A2, 0)
    C2e = st.tile([64, 128], BF, tag="C2e", bufs=3)
    nc.gpsimd.memset(C2e[:, :], 0.0)
    C2o = st.tile([64, 128], BF, tag="C2o", bufs=3)
    nc.gpsimd.memset(C2o[:, :], 0.0)

    Cp = ps.tile([128, 128], F32, tag="Cp", name="Cp")
    emit_cp(Cp, C2e, C2o)

    # ---------------- the recurrence ----------------
    for t in range(NW):
        P0 = ps.tile([128, 256], F32, tag="P0", name="P0", padded_shape=[128, 512])
        P1 = ps.tile([128, 256], F32, tag="P1", name="P1", padded_shape=[128, 512])
        Pu = (P0[:, :], P1[:, :])
        A2h = A2[0:64, :]

        # gate matmuls, u=0 first so sigma-u0 can start early. Each region's
        # rank-1 bias MM sits directly before its A-MM: an intervening MM to
        # another region closes the accumulation group and drops the bias.
        for u in (0, 1):
            Pap = Pu[u]
            for w, LA, LB in ((0, LA0, LB0), (1, LA1, LB1)):
                base = 128 * w
                nc.tensor.matmul(
                    Pap[:, base:base + 128],
                    LBIAS[:, 128 * w:128 * w + 128], ONES[:, 0:128],
                    start=True, stop=False, skip_group_check=True,
                )
                nc.tensor.matmul(
                    Pap[:, base:base + 128], LA[:, :], A2[:, 128 * u:128 * u + 128],
                    start=False, stop=False, skip_group_check=True,
                )
                if u == 1:
                    # out p odd <- h[p-1] (even, same kap)
                    nc.tensor.matmul(
                        Pap[:, base:base + 128], LB[:, :], A2[0:64, 0:128],
                        start=False, stop=True, skip_group_check=True,
                    )
                else:
                    # out p even, kap >= 1 <- h[p-1] (odd, kap-1)
                    nc.tensor.matmul(
                        v(Pap, base + 4, [[1, 124]]),
                        LB[:, :], v(A2h, 128, [[1, 124]]),
                        start=False, stop=True, skip_group_check=True,
                    )

        # upsample + OUT for step t-1 (reads A2 h half = h_{t-1})
        U = None
        if t > 0:
            U = ps.tile([128, 256], F32, tag="U", name="U")
            for u in (0, 1):
                k0, n = parity_band(t - 1, u)
                if n:
                    nc.tensor.matmul(
                        v(U[:, :], 128 * u + 4 * k0, [[1, 4 * n]]),
                        LU[:, :], v(A2h, 128 * u + 4 * k0, [[1, 4 * n]]),
                        start=True, stop=True, skip_group_check=True,
                    )

        # x for step t+1
        A2n = st.tile([128, 256], BF, tag="A2", name="A2n", bufs=3)
        if t + 1 < NW:
            xprep(A2n, t + 1)

        # fused sigmoids: one per parity u, spanning both channel tiles
        G = tmp.tile([128, 512], F32, tag="G")
        Gap = G[:, :]
        for u in (0, 1):
            nc.scalar.activation(
                v(Gap, 8 * u, [[4, 2], [16, 32], [1, 4]]),
                v(Pu[u], 0, [[128, 2], [4, 32], [1, 4]]),
                AF.Sigmoid,
            )

        # gate math, pipelined by parity u across vector/gpsimd/scalar
        T1 = tmp.tile([128, 128], F32, tag="T1")
        T2 = tmp.tile([128, 128], F32, tag="T2")
        C2n = st.tile([128, 128], BF, tag="C2BF", bufs=3)
        TH = tmp.tile([128, 128], F32, tag="TH")
        Cpap = Cp[:, :]
        T1ap, T2ap, C2nap, THap = T1[:, :], T2[:, :], C2n[:, :], TH[:, :]

        # queue-order matters: vector [T2a,ADDa,T2b,ADDb,MUL00,MUL01],
        # gpsimd [T1a,T1b,MUL10,MUL11], scalar [sig0,sig1,TANHa,TANHb] --
        # the u=1 chain pipelines behind u=0 without queue blocking.
        # (gpsimd cannot access PSUM, so T2/OUT stay on vector.)
        for u in (0, 1):
            o = 8 * u
            nc.vector.tensor_mul(
                v(T1ap, o, DU), v(Gap, 0 + o, DU), v(Gap, 128 + o, DU))
            nc.vector.scalar_tensor_tensor(
                out=v(T2ap, o, DU), in0=v(Cpap, o, DU), scalar=bc2c2[:, 0:1],
                in1=v(Gap, 256 + o, DU), op0=ALU.add, op1=ALU.mult)
            nc.vector.tensor_add(v(C2nap, o, DU), v(T1ap, o, DU), v(T2ap, o, DU))
        for u in (0, 1):
            o = 8 * u
            nc.scalar.activation(v(THap, o, DU), v(C2nap, o, DU), AF.Tanh)
        # h = og * tanh(c) into A2n h-half quadrants
        for un in (0, 1):
            for u in (0, 1):
                o = 8 * u
                eng = nc.vector if un == 0 else nc.gpsimd
                eng.tensor_mul(
                    v(A2n[0:64, :], 128 * un + o, DU),
                    v(G[64 * un:64 * un + 64, :], 384 + o, DU),
                    v(TH[64 * un:64 * un + 64, :], o, DU),
                )

        # rebase c halves to partition 0 (vector), then c2c matmuls for t+1
        C2en = st.tile([64, 128], BF, tag="C2e", bufs=3)
        nc.vector.tensor_copy(C2en[:, :], C2n[0:64, :])
        C2on = st.tile([64, 128], BF, tag="C2o", bufs=3)
        nc.vector.tensor_copy(C2on[:, :], C2n[64:128, :])
        Cpn = ps.tile([128, 128], F32, tag="Cp", name="Cpn")
        emit_cp(Cpn, C2en, C2on)

        # OUT bias-add for step t-1 (vector, end of queue) + early row DMA
        if t > 0:
            for u in (0, 1):
                k0, n = parity_band(t - 1, u)
                if n:
                    nc.scalar.add(
                        v(OUT_ap, 63 * (2 * k0 + u) + (t - 1), [[4096, 4], [126, n]]),
                        v(U[:, :], 128 * u + 4 * k0, [[1, 4], [4, n]]),
                        bup[:, 0:1],
                    )
        p_done = t - 64
        if p_done >= 0:
            nc.sync.dma_start(
                out=dv(out_d, p_done * 64, [[4096, 128], [524288, 4], [1, 64]]),
                in_=v(OUT_ap, p_done * 64, [[4096, 4], [1, 64]]),
            )

        A2 = A2n
        Cp = Cpn

    # ---------------- epilogue: upsample/store for t = NW-1 ----------------
    t1 = NW - 1
    A2h = A2[0:64, :]
    U = ps.tile([128, 256], F32, tag="U", name="Ulast")
    for u in (0, 1):
        k0, n = parity_band(t1, u)
        if n:
            nc.tensor.matmul(
                v(U[:, :], 128 * u + 4 * k0, [[1, 4 * n]]),
                LU[:, :], v(A2h, 128 * u + 4 * k0, [[1, 4 * n]]),
                start=True, stop=True, skip_group_check=True,
            )
    for u in (0, 1):
        k0, n = parity_band(t1, u)
        if n:
            nc.vector.tensor_scalar_add(
                v(OUT_ap, 63 * (2 * k0 + u) + t1, [[4096, 4], [126, n]]),
                v(U[:, :], 128 * u + 4 * k0, [[1, 4], [4, n]]),
                bup[:, 0:1],
            )
    nc.sync.dma_start(
        out=dv(out_d, 63 * 64, [[4096, 128], [524288, 4], [1, 64]]),
        in_=v(OUT_ap, 63 * 64, [[4096, 4], [1, 64]]),
    )


def build_nc():
    nc = bacc.Bacc("TRN2", target_bir_lowering=False, debug=False)
    ins = {
        "inputs": nc.dram_tensor("inputs", [B, C, H, W], BF, kind="ExternalInput").ap(),
        "w_i2s": nc.dram_tensor("w_i2s", [4 * HID, C], F32, kind="ExternalInput").ap(),
        "b_i2s": nc.dram_tensor("b_i2s", [4 * HID], F32, kind="ExternalInput").ap(),
        "w_s2s": nc.dram_tensor("w_s2s", [4 * HID, HID, 2], F32, kind="ExternalInput").ap(),
        "b_s2s": nc.dram_tensor("b_s2s", [4 * HID], F32, kind="ExternalInput").ap(),
        "w_c2c": nc.dram_tensor("w_c2c", [HID, HID, 2], F32, kind="ExternalInput").ap(),
        "b_c2c": nc.dram_tensor("b_c2c", [HID], F32, kind="ExternalInput").ap(),
        "w_up": nc.dram_tensor("w_up", [2 * HID, HID], F32, kind="ExternalInput").ap(),
        "b_up": nc.dram_tensor("b_up", [2 * HID], F32, kind="ExternalInput").ap(),
    }
    outs = {"out": nc.dram_tensor("out", [B, 2 * HID, H, W], F32, kind="ExternalOutput").ap()}
    with tile.TileContext(nc) as tc:
        with ExitStack() as ctx:
            build_kernel(ctx, tc, outs, ins)
    nc.compile()
    return nc


# ---------------------------------------------------------------------------
# Harness entry point: full inputs -> shard over 8 cores -> full output.
# ---------------------------------------------------------------------------
import ml_dtypes
from concourse.bass_utils import run_bass_kernel_spmd

N_CORES = 8
TRACE = False
LAST_EXEC_NS = None
_NC = None


def _get_nc():
    global _NC
    if _NC is None:
        _NC = build_nc()
    return _NC


def kernel(**inputs):
    global LAST_EXEC_NS
    nc = _get_nc()
    full = {k: np.ascontiguousarray(np.asarray(val, np.float32))
            for k, val in inputs.items()}
    xs = full["inputs"].astype(ml_dtypes.bfloat16)
    in_maps = []
    for i in range(N_CORES):
        m = dict(full)
        m["inputs"] = np.ascontiguousarray(xs[B * i:B * (i + 1)])
        in_maps.append(m)
    res = run_bass_kernel_spmd(nc, in_maps, list(range(N_CORES)), trace=TRACE)
    LAST_EXEC_NS = res.exec_time_ns
    return np.concatenate([res.results[i]["out"] for i in range(N_CORES)], axis=0)
